# revision 52
# baseline (speedup 1.0000x reference)
"""Causal multi-head attention (B=2, H=16, S=2048, D=64, fp32 I/O) on 8 TRN2
NeuronCores.

Sharding: batch*heads (32 units) split 4-per-core — embarrassingly parallel,
no collectives.

Per-core kernel design (bf16 compute, fp32 PSUM accumulation):
  - scores are computed TRANSPOSED: scoresT[k, q] = K_blk @ Q^T so that the
    softmax numerators P^T[k, q] feed the P@V matmul directly as the moving
    operand (contraction dim k on partitions), with V (natural layout) as the
    stationary operand.
  - A ones-column appended to V accumulates the softmax denominator l[q] in
    the same PSUM accumulation as P@V — no separate reduction pass.
  - Causal masking: off-diagonal blocks are skipped entirely; diagonal blocks
    get a -30000 bias accumulated into PSUM by an extra matmul (stationary =
    identity, moving = strictly-lower-triangular bias matrix) BEFORE exp, so
    exp produces exact zeros there on both exp paths — no post-exp mask op.
  - exp is split across TWO engines to halve the softmax eviction cost:
      * ScalarE: fused PSUM->SBUF exp (scale=1/sqrt(D) folded in).
      * DVE (vector): Schraudolph bit-trick — i16 = round(s*A + B) written as
        int16 then reinterpreted as bf16 gives 2^(s*log2e/8) with ~1.8% rms
        error; fp32->int16 convert saturates, so -30000-masked entries land
        at 0x8000 = -0.0. Routed to balance both engines within each q-half.
  - q is processed in halves of 1024. PSUM: scoresT slots [128,1024]
    (2 banks x 3 bufs) + out^T[65,1024] (2 banks, single-buffered); PV lags
    TWO kj-pairs behind QK so neither exp engine's latency stalls the PE.
  - out^T bank evictions are emitted as soon as each bank's last PV is
    placed; the epilogue tail (reciprocal/scale/store) is DEFERRED into the
    next half's instruction stream so it never delays that half's exp work.
  - Head h+1's casts (Q/K on DVE, V on GpSimd; the Q duplicate is a cheap
    bf16->bf16 re-copy of the cast output) and its bf16 DMA-transposes are
    emitted at the END of head h's FIRST q-half, where the DVE is lightly
    loaded, so the transposed slabs land well before head h+1's first QK.
  - K^T lands in a "paired slab" layout (kj even on partitions 0:64, kj odd
    on 64:128) so paired kj matmuls use disjoint PE row-groups and overlap
    in-array. Head 0 uses 2-piece loads with transposes issued from the
    then-idle ScalarE DMA queue to shorten the preamble.
"""

import numpy as np

import concourse.mybir as mybir
import concourse.tile as tile
from concourse import bacc
from concourse.bass_utils import run_bass_kernel_spmd
from concourse.masks import make_identity, make_lower_triangular

B, H, S, D = 2, 16, 2048, 64
N_CORES = 8
HPC = (B * H) // N_CORES  # heads per core
NT = S // 128  # 16 k/q blocks of 128
FP32 = mybir.dt.float32
BF16 = mybir.dt.bfloat16
I16 = mybir.dt.int16

MASK_BIAS = -30000.0
# Schraudolph constants for exp(s/8) ~= 2^(s*log2e/8) via int16/bf16 bits
SCHRAU_A = float(np.log2(np.e) * 128.0 / 8.0)
SCHRAU_B = 16256.0 - 7.4179

PV_LAG = 2


def use_dve(h, hf, pj, ca):
    """Route this (q-half, kj-pair, chunk) eviction+exp to the DVE
    Schraudolph path, balancing ScalarE and DVE time within each q-half.
    """
    if hf == 0:
        return pj == 1 and ca == 768
    return (pj in (2, 3, 4) and ca == 1024) or (pj == 5 and ca == 1280)


def build_attention():
    nc = bacc.Bacc("TRN2", target_bir_lowering=False)
    q_d = nc.dram_tensor("query", [HPC, S, D], FP32, kind="ExternalInput")
    k_d = nc.dram_tensor("key", [HPC, S, D], FP32, kind="ExternalInput")
    v_d = nc.dram_tensor("value", [HPC, S, D], FP32, kind="ExternalInput")
    o_d = nc.dram_tensor("out", [HPC, S, D], FP32, kind="ExternalOutput")

    with tile.TileContext(nc) as tc:
        with (
            tc.tile_pool(name="singles", bufs=1) as singles,
            tc.tile_pool(name="nat", bufs=4) as nat_pool,
            tc.tile_pool(name="bf", bufs=3) as bf_pool,
            tc.tile_pool(name="slab", bufs=2) as slab_pool,
            tc.tile_pool(name="qt", bufs=2) as qt_pool,
            tc.tile_pool(name="pt", bufs=8) as pt_pool,
            tc.tile_pool(name="ep", bufs=4) as ep_pool,
            tc.tile_pool(name="sc", bufs=3, space="PSUM") as sc_pool,
            tc.tile_pool(name="ops", bufs=1, space="PSUM") as ops_pool,
        ):
            # identity (stationary) + strictly-lower-tri -30000 bias (moving)
            # for the in-PSUM causal mask accumulate on diagonal blocks
            ident = singles.tile([128, 128], BF16, tag="ident")
            make_identity(nc, ident)
            tribias = singles.tile([128, 128], BF16, tag="tribias")
            make_lower_triangular(nc, tribias, val=MASK_BIAS, diag=False)

            hm = NT // 2
            PIECES0 = ((0, 8), (8, 16))  # head 0: pipelined pieces
            PIECES = ((0, 16),)  # steady state: whole-head DMAs

            def load_head(h, pieces, v_between=False):
                natQ = nat_pool.tile([128, NT, D], FP32, tag="natQ", name="natQ")
                natK = nat_pool.tile([128, NT, D], FP32, tag="natK", name="natK")
                qsrc = q_d[h].rearrange("(t p) d -> p t d", p=128)
                ksrc = k_d[h].rearrange("(t p) d -> p t d", p=128)
                natV = None
                for i, (a, b) in enumerate(pieces):
                    nc.sync.dma_start(out=natQ[:, a:b, :], in_=qsrc[:, a:b, :])
                    nc.sync.dma_start(out=natK[:, a:b, :], in_=ksrc[:, a:b, :])
                    if i == 0 and v_between:
                        natV = load_v(h)
                if not v_between:
                    natV = load_v(h)
                return natQ, natK, natV

            def load_v(h):
                natV = nat_pool.tile([128, NT, D], FP32, tag="natV", name="natV")
                vsrc = v_d[h].rearrange("(t p) d -> p t d", p=128)
                nc.sync.dma_start(out=natV, in_=vsrc[:, :, :])
                return natV

            def cast_head(nat, pieces, all_dve=False):
                """bf16 casts (bfQ2 duplicates each 64-col d-block so its
                128-wide transposed chunks land Q^T_j on BOTH partition
                halves). The duplicate is a bf16->bf16 re-copy of the cast
                output (packs at 2x on DVE); V's cast runs on GpSimd."""
                natQ, natK, natV = nat
                bfQ2 = bf_pool.tile(
                    [128, NT, 2, D], BF16, tag="bfQ2", name="bfQ2"
                )
                bfK = bf_pool.tile([128, NT, D], BF16, tag="bfK", name="bfK")
                vaug = bf_pool.tile(
                    [128, NT, D + 1], BF16, tag="vaug", name="vaug"
                )
                v_eng = nc.vector if all_dve else nc.gpsimd
                v_eng.tensor_copy(vaug[:, :, 0:D], natV)
                v_eng.memset(vaug[:, :, D : D + 1], 1.0)
                for sl in (slice(a, b) for a, b in pieces):
                    nc.vector.tensor_copy(bfQ2[:, sl, 0, :], natQ[:, sl, :])
                    # K cast on GpSimd: the kslab transpose it gates has a
                    # half-head of slack, and this frees the DVE
                    (nc.vector if all_dve else nc.gpsimd).tensor_copy(
                        bfK[:, sl, :], natK[:, sl, :]
                    )
                    if all_dve:
                        nc.vector.tensor_copy(bfQ2[:, sl, 1, :], natQ[:, sl, :])
                    else:
                        # bf16->bf16 re-copy of the cast output packs at 2x
                        nc.vector.tensor_copy(
                            bfQ2[:, sl, 1, :], bfQ2[:, sl, 0, :]
                        )
                return bfQ2, bfK, vaug

            def transpose_setup(bfQ2, bfK, pieces, engs=None, into=None):
                """Blocked DMA-transposes: kslab pairs (K^T_{2j} on rows 0:64,
                K^T_{2j+1} on rows 64:128) and q-contiguous qt."""
                if into is not None:
                    kslab, qt3 = into
                else:
                    kslab = slab_pool.tile(
                        [128, NT // 2, 128], BF16, tag="kslab", name="kslab"
                    )
                    qt3 = qt_pool.tile([128, NT, 128], BF16, tag="qt", name="qt")
                bfK_f = bfK.rearrange("p t d -> p (t d)")
                bfQ2_f = bfQ2.rearrange("p t c d -> p (t c d)")
                for i, (j0, j1) in enumerate((a // 2, b // 2) for a, b in pieces):
                    eng = engs[i] if engs else nc.sync
                    eng.dma_start_transpose(
                        out=kslab[:, j0:j1, :],
                        in_=bfK_f[:, j0 * 128 : j1 * 128],
                    )
                    eng.dma_start_transpose(
                        out=qt3[:, 2 * j0 : 2 * j1, :],
                        in_=bfQ2_f[:, j0 * 256 : j1 * 256],
                    )
                return kslab, qt3

            # head 0: loads + casts + transposes emitted up front (transposes
            # ride the then-idle ScalarE DMA queue)
            staged = cast_head(
                load_head(0, PIECES0, v_between=True), PIECES0, all_dve=True
            )
            slabs = transpose_setup(
                staged[0], staged[1], PIECES0, engs=(nc.scalar, nc.scalar)
            )
            deferred_tail = [None]

            for h in range(HPC):
                bfQ2, bfK, vaug = staged
                kslab, qt3 = slabs
                qt = qt3.rearrange("p t i -> p (t i)")
                if h + 1 < HPC:
                    nat_next = load_head(h + 1, PIECES)

                # ---- main loop: q halves x k blocks ----
                for hf in range(2):
                    q0 = 1024 * hf  # absolute start of this q-half
                    q1 = q0 + 1024
                    kj_hi = 8 * (hf + 1)  # kj in [0, kj_hi)
                    # last kj writing each 512-bank of out^T (for stop flags)
                    last_kj = [
                        max(
                            kj
                            for kj in range(kj_hi)
                            if max(q0, 128 * kj) < q0 + 512 * (b + 1)
                        )
                        for b in range(2)
                    ]

                    outps = ops_pool.tile([80, 2, 512], FP32, tag="outps")
                    outps_f = outps.rearrange("p a b -> p (a b)")
                    bfo = ep_pool.tile([80, 1024], BF16, tag="bfo")

                    def emit_pv(pair, qas, chunks):
                        # lane-outer: one V_aug weight load per kj; matmuls
                        # split on the absolute 512 grid (PSUM bank limit)
                        for lane, (kj, qa) in enumerate(zip(pair, qas)):
                            for ca, cb, ptile in chunks:
                                lo = max(ca, qa)
                                while lo < cb:
                                    hi = min(cb, q0 + 512 * ((lo - q0) // 512 + 1))
                                    b = (lo - q0) // 512
                                    nc.tensor.matmul(
                                        outps_f[0:65, lo - q0 : hi - q0],
                                        vaug[:, kj, :],
                                        ptile[:, lane, lo - ca : hi - ca],
                                        start=(kj == 0),
                                        stop=(kj == last_kj[b]),
                                    )
                                    lo = hi
                        for bk in range(2):
                            if pair[0] <= last_kj[bk] <= pair[1]:
                                # bank fully accumulated: evict it now so the
                                # next half's PV can claim it early
                                nc.vector.tensor_copy(
                                    bfo[:, 512 * bk : 512 * bk + 512],
                                    outps_f[0:80, 512 * bk : 512 * bk + 512],
                                )

                    pending = []
                    for pj in range(kj_hi // 2):
                        pair = (2 * pj, 2 * pj + 1)
                        qas = [max(q0, 128 * kj) for kj in pair]
                        # Both lanes of a pair share one PSUM slot and one exp:
                        # the two QK^T matmuls then become ready together (same
                        # WAR release) and execute concurrently in disjoint PE
                        # row-groups.
                        chunks = []
                        for ca in range(qas[0], q1, 512):
                            cb = min(ca + 512, q1)
                            cols = cb - ca
                            slot = sc_pool.tile(
                                [128, 2, 512], FP32, tag="slot", name="slot"
                            )
                            diag_mms = []
                            for lane, (kj, qa) in enumerate(zip(pair, qas)):
                                lo = max(ca, qa)
                                if lo >= cb:
                                    continue
                                rows = (kj % 2) * 64
                                dg = 128 * kj
                                has_diag = ca <= dg < cb
                                nc.tensor.matmul(
                                    slot[:, lane, lo - ca : cols],
                                    kslab[rows : rows + 64, kj // 2, :],
                                    qt[rows : rows + 64, lo:cb],
                                    start=True,
                                    stop=not has_diag,
                                )
                                if has_diag:
                                    diag_mms.append((lane, dg))
                            for lane, dg in diag_mms:
                                # causal mask: accumulate -30000 above the
                                # diagonal (k > q) in PSUM so exp gives exact
                                # 0 on both exp paths
                                nc.tensor.matmul(
                                    slot[:, lane, dg - ca : dg - ca + 128],
                                    ident,
                                    tribias,
                                    start=False,
                                    stop=True,
                                )
                            ptile = pt_pool.tile(
                                [128, 2, 512], BF16, tag="ptile", name="ptile"
                            )
                            # the odd lane's first 128 cols in its diagonal
                            # chunk exp stale PSUM; PV never reads them
                            if cols == 512:
                                p_ap = ptile.rearrange("p a b -> p (a b)")
                                s_ap = slot.rearrange("p a b -> p (a b)")
                            else:
                                p_ap = ptile[:, :, 0:cols]
                                s_ap = slot[:, :, 0:cols]
                            if use_dve(h, hf, pj, ca):
                                # Schraudolph: i16 = round(s*A + B) bits are
                                # bf16 2^(s*log2e/8); saturation maps masked
                                # entries to 0x8000 = -0.0
                                nc.vector.tensor_scalar(
                                    p_ap.bitcast(I16),
                                    s_ap,
                                    SCHRAU_A,
                                    SCHRAU_B,
                                    mybir.AluOpType.mult,
                                    mybir.AluOpType.add,
                                )
                            else:
                                nc.scalar.activation(
                                    p_ap,
                                    s_ap,
                                    mybir.ActivationFunctionType.Exp,
                                    scale=1.0 / np.sqrt(D),
                                )
                            chunks.append((ca, cb, ptile))
                        pending.append((pair, qas, chunks))
                        if pj >= PV_LAG:
                            emit_pv(*pending.pop(0))
                        if pj == (1 if hf == 0 else 3) and deferred_tail[0] is not None:
                            # previous half's epilogue tail: emitted here so
                            # it never delays this half's exp/eviction work
                            deferred_tail[0]()
                            deferred_tail[0] = None
                    for args in pending:
                        emit_pv(*args)

                    # prefetch emission for head h+1 at the END of hf0, where
                    # the DVE is lightly loaded: casts + transposes complete
                    # during hf1, so head h+1's first QK never waits
                    if hf == 0 and h + 1 < HPC:
                        staged = cast_head(nat_next, PIECES)
                        slabs = transpose_setup(staged[0], staged[1], PIECES)

                    # ---- epilogue for this (head, half) ----
                    # bank evictions were emitted inside emit_pv. rows 65:80
                    # copy PSUM garbage; they transpose into columns 65:80 of
                    # onat (never read)
                    onat = ep_pool.tile([128, 8, 80], BF16, tag="onat")
                    last = h == HPC - 1 and hf == 1
                    if last:
                        # tail: transpose per bank so bank 0's transpose
                        # overlaps bank 1's PV/eviction
                        nc.sync.dma_start_transpose(
                            out=onat[:, 0:4, :], in_=bfo[:, 0:512]
                        )
                        nc.sync.dma_start_transpose(
                            out=onat[:, 4:8, :], in_=bfo[:, 512:1024]
                        )
                    else:
                        nc.sync.dma_start_transpose(out=onat, in_=bfo)

                    def tail(onat=onat, h=h, hf=hf):
                        rec = ep_pool.tile([128, 8], FP32, tag="rec")
                        nc.vector.reciprocal(rec, onat[:, :, D])
                        osc = ep_pool.tile([128, 8, D], BF16, tag="osc")
                        for t in range(8):
                            nc.vector.tensor_scalar_mul(
                                osc[:, t, :], onat[:, t, 0:D], rec[:, t : t + 1]
                            )
                        fo = ep_pool.tile([128, 8, D], FP32, tag="fo")
                        # deferred-tail cast on GpSimd: gates only the store
                        nc.gpsimd.tensor_copy(fo, osc)
                        odst = o_d[h].rearrange("(t p) d -> p t d", p=128)
                        nc.sync.dma_start(
                            out=odst[:, 8 * hf : 8 * hf + 8, :], in_=fo
                        )

                    def tail_last(onat=onat, h=h, hf=hf):
                        # per-bank tail, scale-muls split across DVE+ScalarE
                        # (ScalarE is idle here), per-bank stores: minimizes
                        # the serial chain after the final matmul
                        odst = o_d[h].rearrange("(t p) d -> p t d", p=128)
                        osc = ep_pool.tile([128, 8, D], BF16, tag="osc")
                        fo = ep_pool.tile([128, 8, D], FP32, tag="fo")
                        for bk in range(2):
                            t0, t1 = 4 * bk, 4 * bk + 4
                            rec = ep_pool.tile([128, 4], FP32, tag=f"recl{bk}")
                            nc.vector.reciprocal(rec, onat[:, t0:t1, D])
                            for t in range(t0, t1):
                                if t % 2 == 0:
                                    nc.vector.tensor_scalar_mul(
                                        osc[:, t, :],
                                        onat[:, t, 0:D],
                                        rec[:, t - t0 : t - t0 + 1],
                                    )
                                else:
                                    nc.scalar.activation(
                                        osc[:, t, :],
                                        onat[:, t, 0:D],
                                        mybir.ActivationFunctionType.Copy,
                                        scale=rec[:, t - t0 : t - t0 + 1],
                                    )
                            nc.vector.tensor_copy(fo[:, t0:t1, :], osc[:, t0:t1, :])
                            nc.sync.dma_start(
                                out=odst[:, 8 * hf + t0 : 8 * hf + t1, :],
                                in_=fo[:, t0:t1, :],
                            )

                    if last:
                        tail_last()
                    else:
                        deferred_tail[0] = tail

    nc.compile()
    return nc


_NC = None


def _get_nc():
    global _NC
    if _NC is None:
        _NC = build_attention()
    return _NC


def kernel(query, key, value):
    nc = _get_nc()
    q = np.ascontiguousarray(query, dtype=np.float32).reshape(B * H, S, D)
    k = np.ascontiguousarray(key, dtype=np.float32).reshape(B * H, S, D)
    v = np.ascontiguousarray(value, dtype=np.float32).reshape(B * H, S, D)
    in_maps = [
        {
            "query": q[i * HPC : (i + 1) * HPC],
            "key": k[i * HPC : (i + 1) * HPC],
            "value": v[i * HPC : (i + 1) * HPC],
        }
        for i in range(N_CORES)
    ]
    res = run_bass_kernel_spmd(nc, in_maps, core_ids=list(range(N_CORES)))
    out = np.concatenate([res.results[i]["out"] for i in range(N_CORES)], axis=0)
    return out.reshape(B, H, S, D)


# revision 53
# speedup vs baseline: 1.1338x; 1.1338x over previous
"""Causal multi-head attention (B=2, H=16, S=2048, D=64, fp32 I/O) on 8 TRN2
NeuronCores.

Sharding: batch*heads (32 units) split 4-per-core — embarrassingly parallel,
no collectives.

Per-core kernel design (bf16 compute, fp32 PSUM accumulation):
  - scores are computed TRANSPOSED: scoresT[k, q] = K_blk @ Q^T so that the
    softmax numerators P^T[k, q] feed the P@V matmul directly as the moving
    operand (contraction dim k on partitions), with V (natural layout) as the
    stationary operand.
  - A ones-column appended to V accumulates the softmax denominator l[q] in
    the same PSUM accumulation as P@V — no separate reduction pass.
  - Causal masking: off-diagonal blocks are skipped entirely; diagonal blocks
    get a -30000 bias accumulated into PSUM by an extra matmul (stationary =
    identity, moving = strictly-lower-triangular bias matrix) BEFORE exp, so
    exp produces exact zeros there on both exp paths — no post-exp mask op.
  - exp is split across TWO engines to halve the softmax eviction cost:
      * ScalarE: fused PSUM->SBUF exp (scale=1/sqrt(D) folded in).
      * DVE (vector): Schraudolph bit-trick — i16 = round(s*A + B) written as
        int16 then reinterpreted as bf16 gives 2^(s*log2e/8) with ~1.8% rms
        error; fp32->int16 convert saturates, so -30000-masked entries land
        at 0x8000 = -0.0. Routed to balance both engines within each q-half.
  - q is processed in halves of 1024. PSUM: scoresT slots [128,1024]
    (2 banks x 3 bufs) + out^T[65,1024] (2 banks, single-buffered); PV lags
    TWO kj-pairs behind QK so neither exp engine's latency stalls the PE.
  - out^T bank evictions are emitted as soon as each bank's last PV is
    placed; the epilogue tail (reciprocal/scale/store) is DEFERRED into the
    next half's instruction stream so it never delays that half's exp work.
  - Head h+1's casts (Q/K on DVE, V on GpSimd; the Q duplicate is a cheap
    bf16->bf16 re-copy of the cast output) and its bf16 DMA-transposes are
    emitted at the END of head h's FIRST q-half, where the DVE is lightly
    loaded, so the transposed slabs land well before head h+1's first QK.
  - K^T lands in a "paired slab" layout (kj even on partitions 0:64, kj odd
    on 64:128) so paired kj matmuls use disjoint PE row-groups and overlap
    in-array. Head 0 uses 2-piece loads with transposes issued from the
    then-idle ScalarE DMA queue to shorten the preamble.
"""

import numpy as np

import concourse.mybir as mybir
import concourse.tile as tile
from concourse import bacc
from concourse.bass_utils import run_bass_kernel_spmd
from concourse.masks import make_identity, make_lower_triangular

B, H, S, D = 2, 16, 2048, 64
N_CORES = 8
HPC = (B * H) // N_CORES  # heads per core
NT = S // 128  # 16 k/q blocks of 128
FP32 = mybir.dt.float32
BF16 = mybir.dt.bfloat16
I16 = mybir.dt.int16

MASK_BIAS = -30000.0
# Schraudolph constants for exp(s/8) ~= 2^(s*log2e/8) via int16/bf16 bits
SCHRAU_A = float(np.log2(np.e) * 128.0 / 8.0)
SCHRAU_B = 16256.0 - 7.4179

PV_LAG = 2


def use_dve(h, hf, pj, ca):
    """Route this (q-half, kj-pair, chunk) eviction+exp to the DVE
    Schraudolph path, balancing ScalarE and DVE time within each q-half.
    """
    if hf == 0:
        return pj == 0 and ca == 512
    return (pj in (2, 3, 4) and ca == 1024) or (pj == 5 and ca == 1280)


def build_attention():
    nc = bacc.Bacc("TRN2", target_bir_lowering=False)
    q_d = nc.dram_tensor("query", [HPC, S, D], FP32, kind="ExternalInput")
    k_d = nc.dram_tensor("key", [HPC, S, D], FP32, kind="ExternalInput")
    v_d = nc.dram_tensor("value", [HPC, S, D], FP32, kind="ExternalInput")
    o_d = nc.dram_tensor("out", [HPC, S, D], FP32, kind="ExternalOutput")

    with tile.TileContext(nc) as tc:
        with (
            tc.tile_pool(name="singles", bufs=1) as singles,
            tc.tile_pool(name="nat", bufs=4) as nat_pool,
            tc.tile_pool(name="bf", bufs=3) as bf_pool,
            tc.tile_pool(name="slab", bufs=2) as slab_pool,
            tc.tile_pool(name="qt", bufs=2) as qt_pool,
            tc.tile_pool(name="pt", bufs=8) as pt_pool,
            tc.tile_pool(name="ep", bufs=4) as ep_pool,
            tc.tile_pool(name="sc", bufs=3, space="PSUM") as sc_pool,
            tc.tile_pool(name="ops", bufs=1, space="PSUM") as ops_pool,
        ):
            # identity (stationary) + strictly-lower-tri -30000 bias (moving)
            # for the in-PSUM causal mask accumulate on diagonal blocks
            ident = singles.tile([128, 128], BF16, tag="ident")
            make_identity(nc, ident)
            tribias = singles.tile([128, 128], BF16, tag="tribias")
            make_lower_triangular(nc, tribias, val=MASK_BIAS, diag=False)

            hm = NT // 2
            PIECES0 = ((0, 8), (8, 16))  # head 0: pipelined pieces
            PIECES = ((0, 16),)  # steady state: whole-head DMAs

            def load_head(h, pieces, v_between=False):
                natQ = nat_pool.tile([128, NT, D], FP32, tag="natQ", name="natQ")
                natK = nat_pool.tile([128, NT, D], FP32, tag="natK", name="natK")
                qsrc = q_d[h].rearrange("(t p) d -> p t d", p=128)
                ksrc = k_d[h].rearrange("(t p) d -> p t d", p=128)
                natV = None
                for i, (a, b) in enumerate(pieces):
                    nc.sync.dma_start(out=natQ[:, a:b, :], in_=qsrc[:, a:b, :])
                    nc.sync.dma_start(out=natK[:, a:b, :], in_=ksrc[:, a:b, :])
                    if i == 0 and v_between:
                        natV = load_v(h)
                if not v_between:
                    natV = load_v(h)
                return natQ, natK, natV

            def load_v(h):
                natV = nat_pool.tile([128, NT, D], FP32, tag="natV", name="natV")
                vsrc = v_d[h].rearrange("(t p) d -> p t d", p=128)
                nc.sync.dma_start(out=natV, in_=vsrc[:, :, :])
                return natV

            def cast_head(nat, pieces, all_dve=False):
                """bf16 casts (bfQ2 duplicates each 64-col d-block so its
                128-wide transposed chunks land Q^T_j on BOTH partition
                halves). The duplicate is a bf16->bf16 re-copy of the cast
                output (packs at 2x on DVE); V's cast runs on GpSimd."""
                natQ, natK, natV = nat
                bfQ2 = bf_pool.tile(
                    [128, NT, 2, D], BF16, tag="bfQ2", name="bfQ2"
                )
                bfK = bf_pool.tile([128, NT, D], BF16, tag="bfK", name="bfK")
                vaug = bf_pool.tile(
                    [128, NT, D + 1], BF16, tag="vaug", name="vaug"
                )
                v_eng = nc.vector if all_dve else nc.gpsimd
                v_eng.tensor_copy(vaug[:, :, 0:D], natV)
                v_eng.memset(vaug[:, :, D : D + 1], 1.0)
                for sl in (slice(a, b) for a, b in pieces):
                    nc.vector.tensor_copy(bfQ2[:, sl, 0, :], natQ[:, sl, :])
                    # K cast on GpSimd: the kslab transpose it gates has a
                    # half-head of slack, and this frees the DVE
                    (nc.vector if all_dve else nc.gpsimd).tensor_copy(
                        bfK[:, sl, :], natK[:, sl, :]
                    )
                    if all_dve:
                        nc.vector.tensor_copy(bfQ2[:, sl, 1, :], natQ[:, sl, :])
                    else:
                        # bf16->bf16 re-copy of the cast output packs at 2x
                        nc.vector.tensor_copy(
                            bfQ2[:, sl, 1, :], bfQ2[:, sl, 0, :]
                        )
                return bfQ2, bfK, vaug

            def transpose_setup(bfQ2, bfK, pieces, engs=None, into=None):
                """Blocked DMA-transposes: kslab pairs (K^T_{2j} on rows 0:64,
                K^T_{2j+1} on rows 64:128) and q-contiguous qt."""
                if into is not None:
                    kslab, qt3 = into
                else:
                    kslab = slab_pool.tile(
                        [128, NT // 2, 128], BF16, tag="kslab", name="kslab"
                    )
                    qt3 = qt_pool.tile([128, NT, 128], BF16, tag="qt", name="qt")
                bfK_f = bfK.rearrange("p t d -> p (t d)")
                bfQ2_f = bfQ2.rearrange("p t c d -> p (t c d)")
                for i, (j0, j1) in enumerate((a // 2, b // 2) for a, b in pieces):
                    eng = engs[i] if engs else nc.sync
                    eng.dma_start_transpose(
                        out=kslab[:, j0:j1, :],
                        in_=bfK_f[:, j0 * 128 : j1 * 128],
                    )
                    eng.dma_start_transpose(
                        out=qt3[:, 2 * j0 : 2 * j1, :],
                        in_=bfQ2_f[:, j0 * 256 : j1 * 256],
                    )
                return kslab, qt3

            # head 0: loads + casts + transposes emitted up front (transposes
            # ride the then-idle ScalarE DMA queue)
            staged = cast_head(
                load_head(0, PIECES0, v_between=True), PIECES0, all_dve=True
            )
            slabs = transpose_setup(
                staged[0], staged[1], PIECES0, engs=(nc.scalar, nc.scalar)
            )
            deferred_tail = [None]

            for h in range(HPC):
                bfQ2, bfK, vaug = staged
                kslab, qt3 = slabs
                qt = qt3.rearrange("p t i -> p (t i)")
                if h + 1 < HPC:
                    nat_next = load_head(h + 1, PIECES)

                # ---- main loop: q halves x k blocks ----
                for hf in range(2):
                    q0 = 1024 * hf  # absolute start of this q-half
                    q1 = q0 + 1024
                    kj_hi = 8 * (hf + 1)  # kj in [0, kj_hi)
                    # last kj writing each 512-bank of out^T (for stop flags)
                    last_kj = [
                        max(
                            kj
                            for kj in range(kj_hi)
                            if max(q0, 128 * kj) < q0 + 512 * (b + 1)
                        )
                        for b in range(2)
                    ]

                    outps = ops_pool.tile([80, 2, 512], FP32, tag="outps")
                    outps_f = outps.rearrange("p a b -> p (a b)")
                    bfo = ep_pool.tile([80, 1024], BF16, tag="bfo")

                    def emit_pv(pair, qas, chunks):
                        # lane-outer: one V_aug weight load per kj; matmuls
                        # split on the absolute 512 grid (PSUM bank limit)
                        for lane, (kj, qa) in enumerate(zip(pair, qas)):
                            for ca, cb, ptile in chunks:
                                lo = max(ca, qa)
                                while lo < cb:
                                    hi = min(cb, q0 + 512 * ((lo - q0) // 512 + 1))
                                    b = (lo - q0) // 512
                                    nc.tensor.matmul(
                                        outps_f[0:65, lo - q0 : hi - q0],
                                        vaug[:, kj, :],
                                        ptile[:, lane, lo - ca : hi - ca],
                                        start=(kj == 0),
                                        stop=(kj == last_kj[b]),
                                    )
                                    lo = hi
                        for bk in range(2):
                            if pair[0] <= last_kj[bk] <= pair[1]:
                                # bank fully accumulated: evict it now so the
                                # next half's PV can claim it early
                                nc.vector.tensor_copy(
                                    bfo[:, 512 * bk : 512 * bk + 512],
                                    outps_f[0:80, 512 * bk : 512 * bk + 512],
                                )

                    pending = []
                    for pj in range(kj_hi // 2):
                        pair = (2 * pj, 2 * pj + 1)
                        qas = [max(q0, 128 * kj) for kj in pair]
                        # Both lanes of a pair share one PSUM slot and one exp:
                        # the two QK^T matmuls then become ready together (same
                        # WAR release) and execute concurrently in disjoint PE
                        # row-groups.
                        chunks = []
                        for ca in range(qas[0], q1, 512):
                            cb = min(ca + 512, q1)
                            cols = cb - ca
                            slot = sc_pool.tile(
                                [128, 2, 512], FP32, tag="slot", name="slot"
                            )
                            diag_mms = []
                            for lane, (kj, qa) in enumerate(zip(pair, qas)):
                                lo = max(ca, qa)
                                if lo >= cb:
                                    continue
                                rows = (kj % 2) * 64
                                dg = 128 * kj
                                has_diag = ca <= dg < cb
                                nc.tensor.matmul(
                                    slot[:, lane, lo - ca : cols],
                                    kslab[rows : rows + 64, kj // 2, :],
                                    qt[rows : rows + 64, lo:cb],
                                    start=True,
                                    stop=not has_diag,
                                )
                                if has_diag:
                                    diag_mms.append((lane, dg))
                            for lane, dg in diag_mms:
                                # causal mask: accumulate -30000 above the
                                # diagonal (k > q) in PSUM so exp gives exact
                                # 0 on both exp paths
                                nc.tensor.matmul(
                                    slot[:, lane, dg - ca : dg - ca + 128],
                                    ident,
                                    tribias,
                                    start=False,
                                    stop=True,
                                )
                            ptile = pt_pool.tile(
                                [128, 2, 512], BF16, tag="ptile", name="ptile"
                            )
                            # the odd lane's first 128 cols in its diagonal
                            # chunk exp stale PSUM; PV never reads them
                            if cols == 512:
                                p_ap = ptile.rearrange("p a b -> p (a b)")
                                s_ap = slot.rearrange("p a b -> p (a b)")
                            else:
                                p_ap = ptile[:, :, 0:cols]
                                s_ap = slot[:, :, 0:cols]
                            if use_dve(h, hf, pj, ca):
                                # Schraudolph: i16 = round(s*A + B) bits are
                                # bf16 2^(s*log2e/8); saturation maps masked
                                # entries to 0x8000 = -0.0
                                nc.vector.tensor_scalar(
                                    p_ap.bitcast(I16),
                                    s_ap,
                                    SCHRAU_A,
                                    SCHRAU_B,
                                    mybir.AluOpType.mult,
                                    mybir.AluOpType.add,
                                )
                            else:
                                nc.scalar.activation(
                                    p_ap,
                                    s_ap,
                                    mybir.ActivationFunctionType.Exp,
                                    scale=1.0 / np.sqrt(D),
                                )
                            chunks.append((ca, cb, ptile))
                        pending.append((pair, qas, chunks))
                        if pj >= PV_LAG:
                            emit_pv(*pending.pop(0))
                        if pj == (1 if hf == 0 else 3) and deferred_tail[0] is not None:
                            # previous half's epilogue tail: emitted here so
                            # it never delays this half's exp/eviction work
                            deferred_tail[0]()
                            deferred_tail[0] = None
                    for args in pending:
                        emit_pv(*args)

                    # prefetch emission for head h+1 at the END of hf0, where
                    # the DVE is lightly loaded: casts + transposes complete
                    # during hf1, so head h+1's first QK never waits
                    if hf == 0 and h + 1 < HPC:
                        staged = cast_head(nat_next, PIECES)
                        slabs = transpose_setup(staged[0], staged[1], PIECES)

                    # ---- epilogue for this (head, half) ----
                    # bank evictions were emitted inside emit_pv. rows 65:80
                    # copy PSUM garbage; they transpose into columns 65:80 of
                    # onat (never read)
                    onat = ep_pool.tile([128, 8, 80], BF16, tag="onat")
                    last = h == HPC - 1 and hf == 1
                    if last:
                        # tail: transpose per bank so bank 0's transpose
                        # overlaps bank 1's PV/eviction
                        nc.sync.dma_start_transpose(
                            out=onat[:, 0:4, :], in_=bfo[:, 0:512]
                        )
                        nc.sync.dma_start_transpose(
                            out=onat[:, 4:8, :], in_=bfo[:, 512:1024]
                        )
                    else:
                        nc.sync.dma_start_transpose(out=onat, in_=bfo)

                    def tail(onat=onat, h=h, hf=hf):
                        rec = ep_pool.tile([128, 8], FP32, tag="rec")
                        nc.vector.reciprocal(rec, onat[:, :, D])
                        osc = ep_pool.tile([128, 8, D], BF16, tag="osc")
                        for t in range(8):
                            nc.vector.tensor_scalar_mul(
                                osc[:, t, :], onat[:, t, 0:D], rec[:, t : t + 1]
                            )
                        fo = ep_pool.tile([128, 8, D], FP32, tag="fo")
                        # deferred-tail cast on GpSimd: gates only the store
                        nc.gpsimd.tensor_copy(fo, osc)
                        odst = o_d[h].rearrange("(t p) d -> p t d", p=128)
                        nc.sync.dma_start(
                            out=odst[:, 8 * hf : 8 * hf + 8, :], in_=fo
                        )

                    def tail_last(onat=onat, h=h, hf=hf):
                        # per-bank tail, scale-muls split across DVE+ScalarE
                        # (ScalarE is idle here), per-bank stores: minimizes
                        # the serial chain after the final matmul
                        odst = o_d[h].rearrange("(t p) d -> p t d", p=128)
                        osc = ep_pool.tile([128, 8, D], BF16, tag="osc")
                        fo = ep_pool.tile([128, 8, D], FP32, tag="fo")
                        for bk in range(2):
                            t0, t1 = 4 * bk, 4 * bk + 4
                            rec = ep_pool.tile([128, 4], FP32, tag=f"recl{bk}")
                            nc.vector.reciprocal(rec, onat[:, t0:t1, D])
                            for t in range(t0, t1):
                                if t % 2 == 0:
                                    nc.vector.tensor_scalar_mul(
                                        osc[:, t, :],
                                        onat[:, t, 0:D],
                                        rec[:, t - t0 : t - t0 + 1],
                                    )
                                else:
                                    nc.scalar.activation(
                                        osc[:, t, :],
                                        onat[:, t, 0:D],
                                        mybir.ActivationFunctionType.Copy,
                                        scale=rec[:, t - t0 : t - t0 + 1],
                                    )
                            nc.vector.tensor_copy(fo[:, t0:t1, :], osc[:, t0:t1, :])
                            nc.sync.dma_start(
                                out=odst[:, 8 * hf + t0 : 8 * hf + t1, :],
                                in_=fo[:, t0:t1, :],
                            )

                    if last:
                        tail_last()
                    else:
                        deferred_tail[0] = tail

    nc.compile()
    return nc


_NC = None


def _get_nc():
    global _NC
    if _NC is None:
        _NC = build_attention()
    return _NC


def kernel(query, key, value):
    nc = _get_nc()
    q = np.ascontiguousarray(query, dtype=np.float32).reshape(B * H, S, D)
    k = np.ascontiguousarray(key, dtype=np.float32).reshape(B * H, S, D)
    v = np.ascontiguousarray(value, dtype=np.float32).reshape(B * H, S, D)
    in_maps = [
        {
            "query": q[i * HPC : (i + 1) * HPC],
            "key": k[i * HPC : (i + 1) * HPC],
            "value": v[i * HPC : (i + 1) * HPC],
        }
        for i in range(N_CORES)
    ]
    res = run_bass_kernel_spmd(nc, in_maps, core_ids=list(range(N_CORES)))
    out = np.concatenate([res.results[i]["out"] for i in range(N_CORES)], axis=0)
    return out.reshape(B, H, S, D)
